# revision 28
# baseline (speedup 1.0000x reference)
"""Trainium2 Bass kernel: batched bond-angle cosines (gather + vector math).

Problem: geometry (n_atoms, 3, batch) f32, angle triplets (n_angles, 3) int32.
Output: cos(angle) per (triplet, frame) = (n_angles, batch) f32.

Architecture (v4):
- Shard angles across 8 cores (8192 each, 64 tiles of 128 angles).
- Geometry as a (n_atoms, 3*batch) f32 row table (6KB rows). Per tile and
  role, an indirect DGE DMA gathers the 128 endpoint-atom rows into SBUF
  ([x|y|z] planar, 512 frames per coordinate).
- Per tile: d1 = a-b, d2 = c-b on DVE (f32 in, fp16 out: rounding the
  difference keeps the error relative to |v|). Products m = d1*d2 and the
  coordinate-sum adds run in fp16 on DVE (2x perf mode); squares q = d^2 on
  the Scalar engine. The host pre-scales the geometry by 8 so fp16 squares
  of the smallest |v| stay in the normal range while sums stay < 65504
  (cos is scale-invariant). Gathers live on GpSimd.
- Tail: t = n1*n2, s = sqrt(t) (ACT), r ~= 1/s via the custom-DVE
  reciprocal_approx_fast (r(0) = NaN, preserving the reference's 0/0 NaN
  semantics), res = dot*r. Contiguous 128-row DMA to the output.
"""

import numpy as np

import concourse.tile as tile
from concourse import bacc, bass, mybir
from concourse.bass_utils import run_bass_kernel_spmd

P = 128

N_ATOMS = 2048
N_ANGLES = 65536
BATCH = 512
N_CORES = 8
PER_CORE = N_ANGLES // N_CORES  # 8192
N_TILES = PER_CORE // P  # 64

_NC_CACHE = {}


def build_nc(n_atoms=N_ATOMS, per_core=PER_CORE, batch=BATCH):
    n_tiles = per_core // P
    B = batch
    f32 = mybir.dt.float32
    f16 = mybir.dt.float16
    i32 = mybir.dt.int32

    nc = bacc.Bacc(debug=False)

    geom = nc.declare_dram_parameter("geom", [n_atoms, 3 * B], f32, isOutput=False)
    geomn = nc.declare_dram_parameter("geomn", [n_atoms, 3 * B], f32, isOutput=False)
    # idxs[p, t*3 + r] = angles[t*128 + p, r]
    idxs = nc.declare_dram_parameter("idxs", [P, 3 * n_tiles], i32, isOutput=False)
    out = nc.declare_dram_parameter("out", [per_core, B], f32, isOutput=True)

    with tile.TileContext(nc) as tc:
        with (
            tc.tile_pool(name="idxp", bufs=1) as idxp,
            tc.tile_pool(name="gath", bufs=4) as gath,
            tc.tile_pool(name="work", bufs=3) as work,
            tc.tile_pool(name="outp", bufs=3) as outp,
        ):
            idx_sb = idxp.tile([P, 3 * n_tiles], i32)
            nc.sync.dma_start(out=idx_sb[:, :], in_=idxs[:, :])

            W = 2  # tiles per window: phase-ordered so DMA-add chains pipeline

            def _gather(dst, t, role, src, op=mybir.AluOpType.bypass):
                nc.gpsimd.indirect_dma_start(
                    out=dst,
                    out_offset=None,
                    in_=src[:, :],
                    in_offset=bass.IndirectOffsetOnAxis(
                        ap=idx_sb[:, 3 * t + role : 3 * t + role + 1],
                        axis=0,
                    ),
                    compute_op=op,
                )

            for w in range(n_tiles // W):
                v1s, v2s = [], []
                for k in range(W):
                    t = w * W + k
                    v1 = gath.tile([P, 3 * B], f32, tag=f"v1_{k}")
                    v2 = gath.tile([P, 3 * B], f32, tag=f"v2_{k}")
                    v1s.append(v1)
                    v2s.append(v2)
                    _gather(v1[:, :], t, 1, geomn)
                for k in range(W):
                    nc.vector.tensor_copy(v2s[k][:, :], v1s[k][:, :])
                for k in range(W):
                    t = w * W + k
                    _gather(v1s[k][:, :], t, 0, geom, mybir.AluOpType.add)
                    _gather(v2s[k][:, :], t, 2, geom, mybir.AluOpType.add)

                for k in range(W):
                    t = w * W + k
                    v1, v2 = v1s[k], v2s[k]
                    # pk = [m | q1 | q2] packed
                    pk = work.tile([P, 3, 3 * B], f16, tag="pk")
                    nc.vector.tensor_mul(pk[:, 0, :], v1[:, :], v2[:, :])
                    nc.scalar.square(pk[:, 1, :], v1[:, :])
                    nc.scalar.square(pk[:, 2, :], v2[:, :])

                    su = work.tile([P, 3, B], f16, tag="su")
                    t_ = work.tile([P, B], f32, tag="t_")
                    s = work.tile([P, B], f32, tag="s")
                    r = work.tile([P, B], f32, tag="r")
                    nc.vector.tensor_add(
                        su[:, :, :], pk[:, :, 0:B], pk[:, :, B : 2 * B]
                    )
                    nc.vector.tensor_add(
                        su[:, :, :], su[:, :, :], pk[:, :, 2 * B : 3 * B]
                    )
                    dot, n1, n2 = (su[:, i, :] for i in range(3))

                    nc.gpsimd.tensor_mul(t_[:, :], n1, n2)
                    nc.scalar.sqrt(s[:, :], t_[:, :])
                    nc.vector.reciprocal_approx_fast(r[:, :], s[:, :])

                    res = outp.tile([P, B], f32, tag="res")
                    nc.vector.tensor_mul(res[:, :], dot, r[:, :])
                    nc.sync.dma_start(
                        out=out[t * P : (t + 1) * P, :], in_=res[:, :]
                    )

    nc.compile()
    return nc


def _prep_core_inputs(geom2d, angles, core):
    ang = angles[core * PER_CORE : (core + 1) * PER_CORE]
    idxs = np.ascontiguousarray(
        ang.reshape(N_TILES, P, 3).transpose(1, 0, 2).reshape(P, 3 * N_TILES)
    )
    return {"geom": geom2d, "geomn": -geom2d, "idxs": idxs}


def kernel(input, angles, _trace=False, _trace_kwargs=None):
    input = np.ascontiguousarray(np.asarray(input, dtype=np.float32))
    angles = np.ascontiguousarray(np.asarray(angles, dtype=np.int32))
    assert input.shape == (N_ATOMS, 3, BATCH)
    assert angles.shape == (N_ANGLES, 3)

    # scale by 8 so fp16 squares of the smallest nonzero |v| stay normal
    # while |v1|^2 sums stay below fp16 max; cos() is scale-invariant.
    geom2d = (input.reshape(N_ATOMS, 3 * BATCH) * 8.0).astype(np.float32)

    key = (N_ATOMS, PER_CORE, BATCH)
    if key not in _NC_CACHE:
        _NC_CACHE[key] = build_nc(*key)
    nc = _NC_CACHE[key]

    in_maps = [_prep_core_inputs(geom2d, angles, c) for c in range(N_CORES)]
    kw = {}
    if _trace:
        kw["trace"] = True
        kw.update(_trace_kwargs or {})
    res = run_bass_kernel_spmd(nc, in_maps, core_ids=list(range(N_CORES)), **kw)
    outs = [res.results[c]["out"] for c in range(N_CORES)]
    full = np.concatenate(outs, axis=0)
    if _trace:
        return full, res
    return full


# revision 29
# speedup vs baseline: 1.2515x; 1.2515x over previous
"""Trainium2 Bass kernel: batched bond-angle cosines (gather + vector math).

Problem: geometry (n_atoms, 3, batch) f32, angle triplets (n_angles, 3) int32.
Output: cos(angle) per (triplet, frame) = (n_angles, batch) f32.

Architecture (v4):
- Shard angles across 8 cores (8192 each, 64 tiles of 128 angles).
- Geometry as a (n_atoms, 3*batch) f32 row table (6KB rows). Per tile and
  role, an indirect DGE DMA gathers the 128 endpoint-atom rows into SBUF
  ([x|y|z] planar, 512 frames per coordinate).
- Per tile: d1 = a-b, d2 = c-b on DVE (f32 in, fp16 out: rounding the
  difference keeps the error relative to |v|). Products m = d1*d2 and the
  coordinate-sum adds run in fp16 on DVE (2x perf mode); squares q = d^2 on
  the Scalar engine. The host pre-scales the geometry by 8 so fp16 squares
  of the smallest |v| stay in the normal range while sums stay < 65504
  (cos is scale-invariant). Gathers live on GpSimd.
- Tail: t = n1*n2, s = sqrt(t) (ACT), r ~= 1/s via the custom-DVE
  reciprocal_approx_fast (r(0) = NaN, preserving the reference's 0/0 NaN
  semantics), res = dot*r. Contiguous 128-row DMA to the output.
"""

import numpy as np

import concourse.tile as tile
from concourse import bacc, bass, mybir
from concourse.bass_utils import run_bass_kernel_spmd

P = 128

N_ATOMS = 2048
N_ANGLES = 65536
BATCH = 512
N_CORES = 8
PER_CORE = N_ANGLES // N_CORES  # 8192
N_TILES = PER_CORE // P  # 64

_NC_CACHE = {}


def build_nc(n_atoms=N_ATOMS, per_core=PER_CORE, batch=BATCH):
    n_tiles = per_core // P
    B = batch
    f32 = mybir.dt.float32
    f16 = mybir.dt.float16
    i32 = mybir.dt.int32

    nc = bacc.Bacc(debug=False)

    geom = nc.declare_dram_parameter("geom", [n_atoms, 3 * B], f32, isOutput=False)
    # idxs[p, t*3 + r] = angles[t*128 + p, r]
    idxs = nc.declare_dram_parameter("idxs", [P, 3 * n_tiles], i32, isOutput=False)
    out = nc.declare_dram_parameter("out", [per_core, B], f32, isOutput=True)

    with tile.TileContext(nc) as tc:
        with (
            tc.tile_pool(name="idxp", bufs=1) as idxp,
            tc.tile_pool(name="gath", bufs=4) as gath,
            tc.tile_pool(name="work", bufs=3) as work,
            tc.tile_pool(name="outp", bufs=3) as outp,
        ):
            idx_sb = idxp.tile([P, 3 * n_tiles], i32)
            nc.sync.dma_start(out=idx_sb[:, :], in_=idxs[:, :])

            for t in range(n_tiles):
                ga = gath.tile([P, 3 * B], f32, tag="ga")
                gb = gath.tile([P, 3 * B], f32, tag="gb")
                gc = gath.tile([P, 3 * B], f32, tag="gc")
                for role, g in enumerate((ga, gb, gc)):
                    nc.gpsimd.indirect_dma_start(
                        out=g[:, :],
                        out_offset=None,
                        in_=geom[:, :],
                        in_offset=bass.IndirectOffsetOnAxis(
                            ap=idx_sb[:, 3 * t + role : 3 * t + role + 1],
                            axis=0,
                        ),
                    )

                d1 = work.tile([P, 3 * B], f16, tag="d1")
                d2 = work.tile([P, 3 * B], f16, tag="d2")
                # pk = [m | q1 | q2] packed so the three coordinate-sum
                # trees collapse into two wide (128,3,512) adds
                pk = work.tile([P, 3, 3 * B], f16, tag="pk")

                nc.vector.tensor_sub(d1[:, :], ga[:, :], gb[:, :])
                nc.vector.tensor_sub(d2[:, :], gc[:, :], gb[:, :])
                nc.vector.tensor_mul(pk[:, 0, :], d1[:, :], d2[:, :])
                nc.scalar.square(pk[:, 1, :], d1[:, :])
                nc.scalar.square(pk[:, 2, :], d2[:, :])

                # su = [dot | n1 | n2]
                su = work.tile([P, 3, B], f16, tag="su")
                t_ = work.tile([P, B], f32, tag="t_")
                s = work.tile([P, B], f32, tag="s")
                r = work.tile([P, B], f32, tag="r")

                nc.vector.tensor_add(
                    su[:, :, :], pk[:, :, 0:B], pk[:, :, B : 2 * B]
                )
                nc.vector.tensor_add(
                    su[:, :, :], su[:, :, :], pk[:, :, 2 * B : 3 * B]
                )
                dot, n1, n2 = (su[:, i, :] for i in range(3))

                nc.gpsimd.tensor_mul(t_[:, :], n1, n2)
                nc.scalar.sqrt(s[:, :], t_[:, :])
                nc.vector.reciprocal_approx_fast(r[:, :], s[:, :])

                res = outp.tile([P, B], f32, tag="res")
                nc.vector.tensor_mul(res[:, :], dot, r[:, :])
                nc.sync.dma_start(
                    out=out[t * P : (t + 1) * P, :], in_=res[:, :]
                )

    nc.compile()
    return nc


def _prep_core_inputs(geom2d, angles, core):
    ang = angles[core * PER_CORE : (core + 1) * PER_CORE]
    idxs = np.ascontiguousarray(
        ang.reshape(N_TILES, P, 3).transpose(1, 0, 2).reshape(P, 3 * N_TILES)
    )
    return {"geom": geom2d, "idxs": idxs}


def kernel(input, angles, _trace=False, _trace_kwargs=None):
    input = np.ascontiguousarray(np.asarray(input, dtype=np.float32))
    angles = np.ascontiguousarray(np.asarray(angles, dtype=np.int32))
    assert input.shape == (N_ATOMS, 3, BATCH)
    assert angles.shape == (N_ANGLES, 3)

    # scale by 8 so fp16 squares of the smallest nonzero |v| stay normal
    # while |v1|^2 sums stay below fp16 max; cos() is scale-invariant.
    geom2d = (input.reshape(N_ATOMS, 3 * BATCH) * 8.0).astype(np.float32)

    key = (N_ATOMS, PER_CORE, BATCH)
    if key not in _NC_CACHE:
        _NC_CACHE[key] = build_nc(*key)
    nc = _NC_CACHE[key]

    in_maps = [_prep_core_inputs(geom2d, angles, c) for c in range(N_CORES)]
    kw = {}
    if _trace:
        kw["trace"] = True
        kw.update(_trace_kwargs or {})
    res = run_bass_kernel_spmd(nc, in_maps, core_ids=list(range(N_CORES)), **kw)
    outs = [res.results[c]["out"] for c in range(N_CORES)]
    full = np.concatenate(outs, axis=0)
    if _trace:
        return full, res
    return full
